# revision 1
# baseline (speedup 1.0000x reference)
"""Depth-aware 3x3 conv (depth-similarity modulated conv) on 8 Trainium2
NeuronCores, batch-parallel (1 image per core).

out[b,o,h,w] = sum_{c,k} weight[o,c,k] * fd[b,k,h,w] * xpatch[b,c,k,h,w] + bias
fd[k,p] = exp(-8.3 * |depth[p + delta_k] - depth[p]|)   (zero-padded patches)

v4 design (per core, image [64, 256, 256] fp16):
- Padded plane flattened: q = (h+1)*258 + (w+1), NP = 258*258.
- 8 modulated taps in 4 partition-pair tiles (2 taps x 64ch = K=128 matmuls),
  center tap unmodulated K=64. Pair x tiles (xs := x line at XSL+q0-260):
    T1 = [xs@+1 ; xs@+2]   (2 HBM streams)     A=(t0,t1) mb=0 s=0,
                                               B=(t7,t8) mb=516 s=1
    T2 = [xs@+1 ; xs@+257] (DVE 4x copies)     C=(t2,t3) mb=2, D=(t5,t6)
                                               mb=260 (s=0)
  (mb even so DVE tensor_tensor runs 2x; s = column offset at the matmul.)
- fd packed [96, 512] (8 taps x 12 segs; A/B tap-major for DRAM line writes,
  C/D seg-major), sub + in-place abs on DVE, exp on ACT.
- fd fanout to 64 channel rows:
    pairs A,B: DRAM round-trip (4 lines) + 64-partition stride-0 DMAs
    pairs C,D: per-1024-wave PE broadcast matmuls (lhsT = E2 ones-blocks,
      rhs = fdcd half-chunk tiles) -> PSUM, ACT copy -> SBUF fp16. No DMA.
- Modulate: A + C/D on DVE (2x), B on GPSIMD.
- Mains per 512-group: 5 accumulating matmuls, two groups per weight pass;
  ScalarE evicts with bias -> fp16 into upper partitions.
- Cross-chunk software pipeline: chunk i+1's loads/gen/fanout emitted inside
  chunk i's wave loop; input DMAs on the scalar HWDGE ring, fd scratch
  traffic on sync, output stores on vector — so no ring head-of-line stall.
"""
import numpy as np

import concourse.bacc as bacc
import concourse.bass as bass
import concourse.mybir as mybir
import concourse.tile as tile
from concourse.bass_utils import run_bass_kernel_spmd

F16 = mybir.dt.float16
F32 = mybir.dt.float32

B, C, H, W = 8, 64, 256, 256
Hp, Wp = H + 2, W + 2          # 258
NP = Hp * Wp                   # 66564
ALPHA = 8.3

GW = 512                       # matmul group width (psum bank)
SEGS = 12                      # fd segments / groups per chunk
CH = GW * SEGS                 # 6144 output pixels per chunk
NCHUNK = 11                    # 11*6144 = 67584 >= NP
OUTW = NCHUNK * CH
HCH = CH // 2                  # 3072

XSL, XSH = 512, 4608
DSL, DSH = 512, 4608
XW = XSL + NP + XSH
DW = DSL + NP + DSH

T1_W = CH + 518                # 6662
T2_W = CH + 262                # 6406

FD_SL = 512
LW = FD_SL + OUTW + 512        # fd DRAM line width


def _build_nc():
    nc = bacc.Bacc("TRN2", target_bir_lowering=False, debug=False, num_devices=8)
    x_line = nc.declare_dram_parameter("x_line", [C, XW], F16, isOutput=False)
    d_line = nc.declare_dram_parameter("d_line", [1, DW], F32, isOutput=False)
    wts = nc.declare_dram_parameter("wts", [128, 448], F16, isOutput=False)
    bias = nc.declare_dram_parameter("bias", [64, 1], F32, isOutput=False)
    out_l = nc.declare_dram_parameter("out_line", [C, OUTW], F16, isOutput=True)

    x_t = x_line.ap().tensor
    d_t = d_line.ap().tensor
    fd_dram = nc.dram_tensor("fd_scratch", [4, LW], F16)
    fd_t = fd_dram.ap().tensor

    with tile.TileContext(nc) as tc:
        with (
            tc.tile_pool(name="const", bufs=1) as cpool,
            tc.tile_pool(name="xt", bufs=2) as xpool,
            tc.tile_pool(name="fdgen", bufs=2) as gpool,
            tc.tile_pool(name="fr", bufs=2) as fpool,
            tc.tile_pool(name="frsb", bufs=1) as fspool,
            tc.tile_pool(name="mmod", bufs=2) as mpool,
            tc.tile_pool(name="mcd", bufs=2) as mcdpool,
            tc.tile_pool(name="ost", bufs=2) as opool,
            tc.tile_pool(name="ps", bufs=1, space="PSUM") as pspool,
            tc.tile_pool(name="psfr", bufs=2, space="PSUM") as pfpool,
        ):
            wt_sb = cpool.tile([128, 448], F16, tag="w")
            nc.sync.dma_start(wt_sb[:], wts[:])
            bias_sb = cpool.tile([64, 1], F32, tag="b")
            nc.sync.dma_start(bias_sb[:], bias[:])

            # PE warm-up: dependency-free matmuls during the prologue keep
            # the HAM activity window busy so the first real mains run at
            # full clock (PE is otherwise idle for the ~30us front).
            warm = pfpool.tile([128, 2 * GW], F32, name="fps")
            for _ in range(24):
                nc.tensor.matmul(warm[0:64, 0:448], wt_sb[:, 0:64],
                                 wt_sb[:, 0:448], start=True, stop=True)

            def emit_loads(i):
                q0 = i * CH
                xbase = XSL + q0 - 260
                cx = {"i": i, "q0": q0}
                t1 = xpool.tile([128, T1_W], F16, tag="t1", name="t1")
                nc.scalar.dma_start(
                    t1[0:64, :],
                    bass.AP(x_t, xbase + 1, [[XW, 64], [1, T1_W]]))
                nc.scalar.dma_start(
                    t1[64:128, :],
                    bass.AP(x_t, xbase + 2, [[XW, 64], [1, T1_W]]))
                cx["t1"] = t1
                dp = gpool.tile([96, GW], F32, tag="dp", name="dp")
                nc.scalar.dma_start(
                    dp[0:24, :],
                    bass.AP(d_t, DSL + q0 - 259,
                            [[1, 2], [GW, SEGS], [1, GW]]))
                nc.scalar.dma_start(
                    dp[24:48, :],
                    bass.AP(d_t, DSL + q0 + 258,
                            [[1, 2], [GW, SEGS], [1, GW]]))
                for j, dlt in enumerate((-257, -1, 1, 257)):
                    nc.scalar.dma_start(
                        dp[48 + j:96:4, :],
                        bass.AP(d_t, DSL + q0 + dlt, [[GW, SEGS], [1, GW]]))
                dc = gpool.tile([96, GW], F32, tag="dc", name="dc")
                nc.scalar.dma_start(
                    dc[0:48, :],
                    bass.AP(d_t, DSL + q0,
                            [[0, 4], [GW, SEGS], [1, GW]]))
                nc.scalar.dma_start(
                    dc[48:96, :],
                    bass.AP(d_t, DSL + q0,
                            [[GW, SEGS], [0, 4], [1, GW]]))
                cx["dp"], cx["dc"] = dp, dc
                return cx

            def emit_gen(cx):
                i, q0 = cx["i"], cx["q0"]
                df = gpool.tile([96, GW], F32, tag="df", name="df", bufs=1)
                nc.vector.tensor_tensor(df[:], cx["dp"][:], cx["dc"][:],
                                        mybir.AluOpType.subtract)
                da = gpool.tile([96, GW], F32, tag="da", name="da",
                                bufs=1)
                nc.scalar.activation(da[:], df[:],
                                     mybir.ActivationFunctionType.Abs)
                fdp = gpool.tile([96, GW], F16, tag="fdp", name="fdp")
                nc.scalar.activation(fdp[:], da[:],
                                     mybir.ActivationFunctionType.Exp,
                                     scale=-ALPHA)
                for l in range(4):
                    nc.sync.dma_start(
                        bass.AP(fd_t, l * LW + FD_SL + q0,
                                [[GW, SEGS], [1, GW]]),
                        fdp[l * SEGS:(l + 1) * SEGS, :])
                # C/D fd rows -> half-chunk [34, HCH] tiles (C at 0:2, D at
                # 32:34), seg-major source: partition 48+4g+{0:t2,1:t3,2:t5,
                # 3:t6}
                fdcds = []
                for h in range(2):
                    fdcd = gpool.tile([34, HCH], F16, tag="fdcd", bufs=3,
                                      name=f"fdcd{h}")
                    p0 = 48 + h * 24
                    for j, r in enumerate((0, 1, 32, 33)):
                        nc.sync.dma_start(
                            fdcd[r:r + 1, :],
                            fdp[p0 + j:p0 + 24:4, :])
                    fdcds.append(fdcd)
                cx["fdcd"] = fdcds
                return cx

            def emit_x2_fr(cx):
                q0 = cx["q0"]
                t1 = cx["t1"]
                t2 = xpool.tile([128, T2_W], F16, tag="t2", name="t2")
                nc.vector.tensor_copy(t2[0:64, :], t1[0:64, 0:T2_W])
                nc.vector.tensor_copy(t2[64:128, :],
                                      t1[0:64, 256:256 + T2_W])
                cx["t2"] = t2
                frA = fpool.tile([128, CH], F16, tag="frA", name="frA")
                for l in range(2):
                    nc.sync.dma_start(
                        frA[64 * l:64 * (l + 1), :],
                        bass.AP(fd_t, l * LW + FD_SL + q0,
                                [[0, 64], [1, CH]]))
                frB = fpool.tile([128, CH + 2], F16, tag="frB", name="frB")
                for l in range(2):
                    nc.sync.dma_start(
                        frB[64 * l:64 * (l + 1), :],
                        bass.AP(fd_t, (2 + l) * LW + FD_SL + q0 - 1,
                                [[0, 64], [1, CH + 2]]))
                cx["frA"], cx["frB"] = frA, frB
                return cx

            def emit_modAB(cx):
                mtA, mtB = [], []
                for h in range(2):
                    ma = mpool.tile([128, HCH], F16, tag=f"mtA{h}",
                                    name=f"mtA{h}")
                    mtA.append(ma)
                    mb = mpool.tile([128, HCH + 2], F16, tag=f"mtB{h}",
                                    name=f"mtB{h}")
                    mtB.append(mb)
                cx["mtA"], cx["mtB"] = mtA, mtB
                cx["mcds"] = {}
                cx["osts"] = [opool.tile([128, HCH], F16, tag="o",
                                         name=f"ost{h}") for h in range(2)]
                return cx

            def wave_modAB(cx, w):
                # modulate pair A (DVE) and B (GPSIMD) for wave w's two
                # groups; B's last wave per half carries the +2 tail.
                t1 = cx["t1"]
                h, wl = w // 3, w % 3
                ma = cx["mtA"][h]
                nc.vector.tensor_tensor(
                    ma[:, wl * 1024:(wl + 1) * 1024],
                    t1[:, h * HCH + wl * 1024:h * HCH + (wl + 1) * 1024],
                    cx["frA"][:, h * HCH + wl * 1024:
                              h * HCH + (wl + 1) * 1024],
                    mybir.AluOpType.mult)
                bw = 1026 if wl == 2 else 1024
                mb = cx["mtB"][h]
                nc.gpsimd.tensor_tensor(
                    mb[:, wl * 1024:wl * 1024 + bw],
                    t1[:, 516 + h * HCH + wl * 1024:
                       516 + h * HCH + wl * 1024 + bw],
                    cx["frB"][:, h * HCH + wl * 1024:
                              h * HCH + wl * 1024 + bw],
                    mybir.AluOpType.mult)

            def mains2(cx, g0):
                t1, mtA, mtB = cx["t1"], cx["mtA"], cx["mtB"]
                ps2 = pspool.tile([64, 2 * GW], F32, name="ps2", bufs=2)
                pss = [ps2[:, 0:GW], ps2[:, GW:2 * GW]]
                rhss = []
                for g in (g0, g0 + 1):
                    h, lo = (0, 0) if g < 6 else (1, HCH)
                    mc, md = cx["mcds"].pop(g)
                    rhss.append((
                        mtA[h][:, g * GW - lo:(g + 1) * GW - lo],
                        mtB[h][:, 1 + g * GW - lo:1 + (g + 1) * GW - lo],
                        mc, md,
                        t1[0:64, 259 + g * GW: 259 + (g + 1) * GW]))

                for blk in range(5):
                    lhs = (wt_sb[:, blk * 64:(blk + 1) * 64] if blk < 4
                           else wt_sb[0:64, 256:320])
                    for j in range(2):
                        nc.tensor.matmul(
                            pss[j], lhs, rhss[j][blk],
                            start=(blk == 0), stop=(blk == 4))
                h, lo = (0, 0) if g0 < 6 else (1, HCH)
                nc.scalar.activation(
                    cx["osts"][h][64:128, g0 * GW - lo:(g0 + 2) * GW - lo],
                    ps2[:],
                    mybir.ActivationFunctionType.Identity,
                    bias=bias_sb[:], scale=1.0)

            # ---- software pipeline over chunks ----
            cur = emit_modAB(emit_x2_fr(emit_gen(emit_loads(0))))
            for i in range(NCHUNK):
                q0 = cur["q0"]
                nxt = None
                for w in range(SEGS // 2):
                    fdcd = cur["fdcd"][w // 3]
                    wl = w % 3
                    for pj, (pbase, mb0) in enumerate(((0, 2), (32, 260))):
                        fps = pfpool.tile([128, 2 * GW], F32, name="fps")
                        for gg in range(2):
                            nc.tensor.matmul(
                                fps[:, gg * GW:(gg + 1) * GW],
                                wt_sb[pbase:pbase + 2, 320:448],
                                fdcd[pbase:pbase + 2,
                                     (2 * wl + gg) * GW:
                                     (2 * wl + gg + 1) * GW],
                                start=True, stop=True,
                                tile_position=(pbase, 0))
                        fsb = fspool.tile([128, 2 * GW], F16, tag=f"f{pj}",
                                          name=f"fsb{pj}")
                        nc.scalar.activation(
                            fsb[:], fps[:],
                            mybir.ActivationFunctionType.Identity)
                        mm = mcdpool.tile([128, 2 * GW], F16, tag=f"m{pj}",
                                          name=f"mm{pj}")
                        nc.vector.tensor_tensor(
                            mm[:],
                            cur["t2"][:, mb0 + w * 2 * GW:
                                      mb0 + (w + 1) * 2 * GW],
                            fsb[:], mybir.AluOpType.mult)
                        for gg in range(2):
                            cur["mcds"].setdefault(
                                2 * w + gg, [None, None])[pj] = \
                                mm[:, gg * GW:(gg + 1) * GW]
                    wave_modAB(cur, w)
                    if w > 0:
                        mains2(cur, 2 * w - 2)
                    if i + 1 < NCHUNK:
                        if w == 1:
                            nxt = emit_loads(i + 1)
                        elif w == 2:
                            emit_gen(nxt)
                        elif w == 4:
                            emit_x2_fr(nxt)
                        elif w == 5:
                            emit_modAB(nxt)
                    if w == 3:
                        nc.scalar.dma_start(
                            out_l[:, q0:q0 + HCH],
                            cur["osts"][0][64:128, :])
                mains2(cur, 10)
                nc.scalar.dma_start(
                    out_l[:, q0 + HCH:q0 + CH], cur["osts"][1][64:128, :])
                cur = nxt
    nc.compile()
    return nc


_NC_CACHE = None


def _get_nc():
    global _NC_CACHE
    if _NC_CACHE is None:
        _NC_CACHE = _build_nc()
    return _NC_CACHE


def _make_in_maps(inputs):
    x = np.asarray(inputs["x"], dtype=np.float32)
    depth = np.asarray(inputs["depth"], dtype=np.float32)
    weight = np.asarray(inputs["weight"], dtype=np.float32)
    bias_np = np.asarray(inputs["bias"], dtype=np.float32)

    xl = np.zeros((B, C, XW), np.float16)
    xpad = np.zeros((B, C, Hp, Wp), np.float32)
    xpad[:, :, 1:257, 1:257] = x
    xl[:, :, XSL:XSL + NP] = xpad.reshape(B, C, NP).astype(np.float16)

    dl = np.zeros((B, 1, DW), np.float32)
    dpad = np.zeros((B, Hp, Wp), np.float32)
    dpad[:, 1:257, 1:257] = depth[:, 0]
    dl[:, 0, DSL:DSL + NP] = dpad.reshape(B, NP)

    wts = np.zeros((128, 448), np.float16)
    # pairs: A=(t0,t1), B=(t7,t8), C=(t2,t3), D=(t5,t6); lhsT[c,o] = w[o,c,k]
    for g, (ta, tb) in enumerate(((0, 1), (7, 8), (2, 3), (5, 6))):
        wts[0:64, g * 64:(g + 1) * 64] = \
            weight[:, :, ta // 3, ta % 3].T.astype(np.float16)
        wts[64:128, g * 64:(g + 1) * 64] = \
            weight[:, :, tb // 3, tb % 3].T.astype(np.float16)
    wts[0:64, 256:320] = weight[:, :, 1, 1].T.astype(np.float16)
    # E2 broadcast matrices (C at partitions 0:2, D at 32:34):
    # psum rows 0:64 <- rhs row 0, rows 64:128 <- rhs row 1
    wts[0, 320:384] = 1.0
    wts[1, 384:448] = 1.0
    wts[32, 320:384] = 1.0
    wts[33, 384:448] = 1.0

    bias_col = bias_np.reshape(64, 1)
    return [
        {"x_line": xl[b], "d_line": dl[b], "wts": wts, "bias": bias_col}
        for b in range(B)
    ]


def kernel(x, depth, weight, bias):
    nc = _get_nc()
    in_maps = _make_in_maps(
        {"x": x, "depth": depth, "weight": weight, "bias": bias})
    res = run_bass_kernel_spmd(nc, in_maps, list(range(B)))

    out = np.empty((B, C, H, W), np.float32)
    for b in range(B):
        ol = res.results[b]["out_line"][:, :NP].astype(np.float32)
        out[b] = ol.reshape(C, Hp, Wp)[:, 1:257, 1:257]
    return out

